# revision 22
# baseline (speedup 1.0000x reference)
"""DWT roundtrip (Haar wavedec2 x2 + band downsample -> cubic upsample + waverec2)
as a fused single-pass Trainium2 kernel.

Math: for input x, the reference computes
  aa1, lh1, hl1, hh1 = haar_dwt2(x)            # level-1 bands, [H/2, W/2]
  z = stack(haar_dwt2(aa1), area2(lh1), area2(hl1), area2(hh1))
  ...decode: aa1r = idwt2(dwt2(aa1)) == aa1 (exact roundtrip), and
  out = haar_idwt2(aa1, U(D(lh1)), U(D(hl1)), U(D(hh1)))
with U = cv2-cubic 2x upsample, D = 2x2 box mean. Everything is linear and
local, so the level-2 roundtrip cancels analytically and the whole model is

  out[2i+p, 2j+q] = P[i,j]/4 + UG_pq[i,j]        (p,q in {0,1})

where P = 2x2 block sums of x (== 2*aa1) and UG_pq = U_w(G_pq) with
  G_pq = (-1)^p DQ_lh + (-1)^q DQ_hl + (-1)^(p+q) DQ_hh,
DQ_b the 4x4-block Haar-detail sums of x (== 8*D(band_b)), and U_w the 2D cubic
upsample with the 1/16 normalization folded into its column matrix.

Layout: one image [512,512] per step; SBUF tile [128 partitions x 2048], each
partition owns 4 consecutive image rows, so every row/col pair op is a
free-axis DVE op. The quarter-res -> half-res cubic upsample runs on the
TensorEngine as Zt = G^T @ Ah^T (one matmul, no transposes needed) followed by
UG = (Zt-slice)^T @ Aw^T, and the P/4 term is accumulated into the same PSUM
via a single full-width 0.25*I matmul (P's par-major column layout matches
ug's). ScalarE interleaves PSUM results into the output tile.

VectorE is the bottleneck engine (wall-to-wall busy in the baseline trace), so
the detail chain is tuned for DVE perf modes:
 - detail ops run in bf16: contiguous tensor_tensor hits the 2x_1P packed mode
   (strided pair-ops stay 1x — stride-2 reads can't pack);
 - tensor_reduce is 1x-always on cayman, so the column reduction is a pair-sum
   tree of tensor_tensor ops in one scratch tile U, with the three level-2
   pair-sums ([q1|s1|c2] -> [DQhl|DQlh|DQhh]) merged into ONE strided TT;
 - the Hadamard combos read/write U slices placed so every op is contiguous.
Total per-image DVE: 12 ops, ~5.6us (vs ~6.9us fp32 baseline).

All four constants ship in one inline DRAM tensor (bf16 bytes, the fp32
identity stashed as 2x bf16 words) -> one DMA, issued after image 0's load so
the first RS isn't delayed by const transfers.

Sharding: pure data-parallel, batch 32 -> 4 samples (12 images) per core.
"""

import numpy as np

import concourse.bass as bass
import concourse.mybir as mybir
from concourse import tile
from concourse.bass_utils import run_bass_kernel_spmd

N_CORES = 8
B, C, H, W = 32, 3, 512, 512
IMGS_PER_CORE = (B // N_CORES) * C  # 12

F32 = mybir.dt.float32
F32R = mybir.dt.float32r
BF16 = mybir.dt.bfloat16
ADD = mybir.AluOpType.add
SUB = mybir.AluOpType.subtract

WE = (-0.03515625, 0.26171875, 0.87890625, -0.10546875)

# float32r streams 4x faster than float32 through the PE at N>=256 with
# near-fp32 accuracy (used for the aa-term identity matmuls and, when
# DETAIL_DT=F32, the upsample matmuls too).
MM_DT = F32R
DETAIL_DT = BF16  # ~4.6e-3 rel err, inside the 2e-2 gate; unlocks DVE 2x mode

# U scratch layout (element offsets, det_dt): level-1 pair fields then the
# level-2 DQ bands interleaved with the Hadamard slots so [s|lh|d] is
# contiguous for the GA/GB combo ops.
Q1_OFF, S1_OFF, C2_OFF = 0, 256, 512
DQHL, S_OFF, DQLH, D_OFF, DQHH = 768, 896, 1024, 1152, 1280
U_W = 1408


def _build_A(n):
    """Cubic 2x upsample matrix [2n, n]: out = A @ q along an axis,
    edge-replicated like cv2 (weights accumulate on clamped taps)."""
    A = np.zeros((2 * n, n), dtype=np.float64)
    Wr = (WE[3], WE[2], WE[1], WE[0])
    for u in range(n):
        for t in range(4):
            A[2 * u, min(max(u - 2 + t, 0), n - 1)] += WE[t]
            A[2 * u + 1, min(max(u - 1 + t, 0), n - 1)] += Wr[t]
    return A


def _legalize_waits(nc):
    """This walrus build accepts at most one sync wait per instruction; Tile
    occasionally emits more (notably the kernel-tail DMA drain). Hoist extra
    waits onto standalone EventSemaphore instructions placed just before."""
    for f in nc.m.functions:
        for blk in f.blocks:
            new = []
            changed = False
            for inst in blk.instructions:
                si = inst.sync_info
                if si is not None and len(si.on_wait) > 1:
                    waits = list(si.on_wait)
                    for k, w in enumerate(waits[:-1]):
                        ev = mybir.InstEventSemaphore(
                            name=f"{inst.name}_hw{k}",
                            ins=[],
                            outs=[],
                            engine=inst.engine,
                            sync_info=mybir.SyncInfo(on_wait=[w], on_update=[]),
                        )
                        new.append(ev)
                    inst.sync_info = mybir.SyncInfo(
                        on_wait=[waits[-1]], on_update=list(si.on_update)
                    )
                    changed = True
                new.append(inst)
            if changed:
                blk.instructions = new


def build_nc(n_imgs=IMGS_PER_CORE, mm_dt=MM_DT, det_dt=DETAIL_DT, legalize=True):
    nc = bass.Bass(trn_type="TRN2", target_bir_lowering=False, debug=False)

    x = nc.dram_tensor("x", [n_imgs, H, W], F32, kind="ExternalInput").ap()
    y = nc.dram_tensor("y", [n_imgs, H, W], F32, kind="ExternalOutput").ap()

    # dtype of the upsample (G-band) matmuls follows the detail chain
    g_dt = det_dt if det_dt == BF16 else mm_dt

    A = _build_A(128)
    # AhT[k, n]: n<128 -> even half-rows A[2n,k]; n>=128 -> odd half-rows.
    AhT = np.concatenate([A[0::2, :].T, A[1::2, :].T], axis=1).astype(np.float32)
    AwT = (A.T / 16.0).astype(np.float32)  # [128, 256], natural col order
    np_g = mybir.dt.np(g_dt)
    np_i4 = (0.25 * np.eye(128)).astype(np.float32)
    # One inline tensor = one const DMA: [ahT | awT | -awT | i4-as-g_dt-words].
    gw = 1 if g_dt == F32R else 2  # g_dt words per fp32
    cw = 3 * 256 + 128 * gw
    cnp = np.zeros((128, cw), dtype=np_g)
    cnp[:, 0:256] = AhT.astype(np_g)
    cnp[:, 256:512] = AwT.astype(np_g)
    cnp[:, 512:768] = (-AwT).astype(np_g)
    cnp[:, 768:cw] = np_i4.view(np.uint32 if gw == 1 else np.uint16).view(np_g)
    c_d = nc.inline_tensor(np.ascontiguousarray(cnp), name="consts").ap()

    with tile.TileContext(nc) as tc:
        with (
            tc.tile_pool(name="const", bufs=1) as cpool,
            tc.tile_pool(name="io", bufs=2) as iop,
            tc.tile_pool(name="work", bufs=4) as wp,
            tc.tile_pool(name="psum", bufs=1, space="PSUM") as pzt,
            tc.tile_pool(name="psug", bufs=1, space="PSUM") as pug,
        ):
            ct = cpool.tile([128, cw], g_dt, tag="consts")
            ahT, awT, awTn = ct[:, 0:256], ct[:, 256:512], ct[:, 512:768]
            i4_r = ct[:, 768:cw].bitcast(mm_dt)

            def g_cast(ap):
                return ap if det_dt == BF16 else ap.bitcast(mm_dt)

            import bass_rust as _br

            # Single-image steps at both ends (head: first RS only waits one
            # 1MiB load; tail: pipeline drains one image deep, not two), image
            # pairs in the middle: one DVE instruction covers both images of a
            # pair, amortizing the ~66-cycle per-op init overhead.
            # iofs staggers the PSUM slot tags of consecutive single-image
            # steps so their PE stages don't serialize on slot reuse.
            steps = (
                [(0, 1, 0), (1, 1, 1)]
                + [(2 + 2 * j, 2, 0) for j in range((n_imgs - 4) // 2)]
                + [(n_imgs - 2, 1, 0), (n_imgs - 1, 1, 1)]
            )
            for k, (m0, nm, iofs) in enumerate(steps):
                # ---- load images: partition p <- rows 4p..4p+3, m-major ----
                X = iop.tile([128, 4096], F32, tag="xin")
                nc.sync.dma_start(
                    out=X[:, 0 : nm * 2048].rearrange("p (m z) -> p m z", m=nm),
                    in_=x[m0 : m0 + nm].rearrange("m (p r) w -> p m (r w)", p=128),
                )
                if k == 0:
                    # consts go out on the scalar HWDGE ring so they overlap
                    # image 0's load on the sync ring; needed first by the
                    # zt matmuls which start ~4us into image 0's compute
                    nc.scalar.dma_start(out=ct, in_=c_d.bitcast(g_dt))
                X4 = X[:, 0 : nm * 2048].rearrange("p (m r w) -> p m r w", m=nm, r=4)

                # ---- aa path (fp32): rs = row pairs, P = 2x2 block sums ----
                RS = wp.tile([128, 2048], F32, tag="rs")
                RS4 = RS[:, 0 : nm * 1024].rearrange("p (m r w) -> p m r w", m=nm, r=2)
                nc.vector.tensor_tensor(
                    out=RS4, in0=X4[:, :, 0::2, :], in1=X4[:, :, 1::2, :], op=ADD
                )
                P = wp.tile([128, 1024], F32, tag="p")
                Pb = P.bitcast(mm_dt)
                Pb4 = Pb[:, 0 : nm * 512].rearrange("p (m r w) -> p m r w", m=nm, r=2)
                # P runs on GpSimd: slow (~2.5 cyc/elem) but off the DVE
                # critical path — its consumer (the identity matmuls) runs
                # ~4us later, so the latency hides in slack.
                nc.gpsimd.tensor_tensor(
                    out=Pb4, in0=RS4[:, :, :, 0::2], in1=RS4[:, :, :, 1::2], op=ADD
                )

                # ---- detail path (det_dt): e/o -> rss/rdd ----
                # One TT makes both quarter-row cross sums:
                #   e = x[4u]+x[4u+2], o = x[4u+1]+x[4u+3]
                EO = wp.tile([128, 2048], det_dt, tag="eo")
                EO4 = EO[:, 0 : nm * 1024].rearrange("p (m r w) -> p m r w", m=nm, r=2)
                nc.vector.tensor_tensor(
                    out=EO4, in0=X4[:, :, 0:2, :], in1=X4[:, :, 2:4, :], op=ADD
                )
                # rss/rdd per image; all contiguous bf16 -> DVE 2x mode
                RSD = wp.tile([128, 2048], det_dt, tag="rsd")
                RSD4 = RSD[:, 0 : nm * 1024].rearrange("p (m r w) -> p m r w", m=nm, r=2)
                nc.vector.tensor_tensor(
                    out=RSD4[:, :, 0, :], in0=EO4[:, :, 0, :], in1=EO4[:, :, 1, :], op=ADD
                )
                nc.vector.tensor_tensor(
                    out=RSD4[:, :, 1, :], in0=EO4[:, :, 0, :], in1=EO4[:, :, 1, :], op=SUB
                )
                RSDp = RSD[:, 0 : nm * 1024].rearrange(
                    "p (m r v k) -> p m r v k", m=nm, r=2, k=2
                )

                # ---- column reduction as a pair-sum tree in U ----
                # per-image U block (1536 elems, 128-granular slots):
                #  [q1(2) | s1(2) | c2(2) | hl | s | lh | d | hh | pad(2)]
                U = wp.tile([128, 3072], det_dt, tag="u")
                Us = U[:, 0 : nm * 1536]
                V128 = Us.rearrange("p (m a w) -> p m a w", m=nm, a=12)
                V256 = Us.rearrange("p (m a w) -> p m a w", m=nm, a=6)
                Um = Us.rearrange("p (m v k) -> p m v k", m=nm, v=768, k=2)
                # level 1: q1 = pairdiff(rss), c2 = pairdiff(rdd) (one TT),
                #          s1 = pairsum(rdd)
                nc.vector.tensor_tensor(
                    out=V256[:, :, 0:3:2, :],
                    in0=RSDp[:, :, :, :, 0],
                    in1=RSDp[:, :, :, :, 1],
                    op=SUB,
                )
                nc.vector.tensor_tensor(
                    out=V256[:, :, 1, :],
                    in0=RSDp[:, :, 1, :, 0],
                    in1=RSDp[:, :, 1, :, 1],
                    op=ADD,
                )
                # level 2, one merged TT: [q1|s1|c2] pairs -> [DQhl|DQlh|DQhh]
                # scattered to slots 6/8/10 so lh lands between s and d
                nc.vector.tensor_tensor(
                    out=V128[:, :, 6:11:2, :],
                    in0=Um[:, :, 0:384, 0],
                    in1=Um[:, :, 0:384, 1],
                    op=ADD,
                )

                # ---- Hadamard combos (all contiguous bf16 slices of U) ----
                #   s = hl+hh, d = hl-hh; [s|lh|d] contiguous =>
                #   GA = [lh+s | lh+d] = [G00 | -G11], GB = [lh-s | d-lh] = [G01 | G10]
                nc.vector.tensor_tensor(
                    out=V128[:, :, 7, :], in0=V128[:, :, 6, :], in1=V128[:, :, 10, :], op=ADD
                )
                nc.vector.tensor_tensor(
                    out=V128[:, :, 9, :], in0=V128[:, :, 6, :], in1=V128[:, :, 10, :], op=SUB
                )
                dql2 = _br.AP(
                    tensor=U.tensor,
                    offset=U.offset + 1024,
                    ap=[list(U.ap[0]), [1536, nm], [0, 2], [1, 128]],
                )
                GA = wp.tile([128, 512], det_dt, tag="ga")
                GA4 = GA[:, 0 : nm * 256].rearrange("p (m a w) -> p m a w", m=nm, a=2)
                nc.vector.tensor_tensor(
                    out=g_cast(GA4), in0=dql2, in1=V128[:, :, 7:10:2, :], op=ADD
                )
                GB = wp.tile([128, 512], det_dt, tag="gb")
                GB4 = GB[:, 0 : nm * 256].rearrange("p (m a w) -> p m a w", m=nm, a=2)
                nc.vector.tensor_tensor(
                    out=g_cast(GB4), in0=V128[:, :, 8:10, :], in1=V128[:, :, 7:9, :], op=SUB
                )

                Xo = iop.tile([128, 4096], F32, tag="xout")
                for ii in range(nm):
                    i = ii + iofs if nm == 1 else ii
                    # bands b0..b3 = G00, G01, G10, G11' (b3 negated; AwTn
                    # compensates)
                    go = ii * 256
                    G = [
                        GA[:, go : go + 128],
                        GB[:, go : go + 128],
                        GB[:, go + 128 : go + 256],
                        GA[:, go + 128 : go + 256],
                    ]

                    # ---- cubic upsample on PE: Zt_b = G_b^T @ AhT ----
                    zts = []
                    for pair in range(2):
                        zt_ps = pzt.tile([128, 512], F32, tag=f"zt{pair}_{i}")
                        for half in range(2):
                            nc.tensor.matmul(
                                out=zt_ps[:, half * 256 : half * 256 + 256],
                                lhsT=g_cast(G[2 * pair + half]),
                                rhs=ahT,
                                start=True,
                                stop=True,
                            )
                        zt_sb = wp.tile([128, 512], g_dt, tag=f"ztsb{pair}_{i}")
                        nc.scalar.copy(
                            out=g_cast(zt_sb) if det_dt != BF16 else zt_sb, in_=zt_ps
                        )
                        zts.append(zt_sb)

                    # ---- UG_pq = Zt-slice^T @ AwT (+ 0.25*I @ P), interleave ----
                    # P's (par, 256) column layout matches ug's, so one
                    # full-width identity matmul seeds both parity halves.
                    Xo3 = Xo[:, ii * 2048 : (ii + 1) * 2048].rearrange(
                        "p (r w) -> p r w", r=4
                    )
                    for bi, (p_par, q_par) in enumerate([(0, 0), (0, 1), (1, 0), (1, 1)]):
                        zt_sb = zts[bi // 2]
                        zoff = (bi % 2) * 256
                        rhs = awTn if bi == 3 else awT
                        ug = pug.tile([128, 512], F32, tag=f"ug{bi % 2}_{i}")
                        nc.tensor.matmul(
                            out=ug,
                            lhsT=i4_r,
                            rhs=Pb[:, ii * 512 : (ii + 1) * 512],
                            start=True,
                            stop=False,
                        )
                        for par in range(2):  # half-row parity: ev, od
                            sl = slice(par * 256, par * 256 + 256)
                            lhsT = zt_sb[:, zoff + par * 128 : zoff + par * 128 + 128]
                            nc.tensor.matmul(
                                out=ug[:, sl],
                                lhsT=g_cast(lhsT),
                                rhs=rhs,
                                start=False,
                                stop=True,
                            )
                        ug3 = ug.rearrange("p (a b) -> p a b", a=2)
                        nc.scalar.copy(out=Xo3[:, p_par::2, q_par::2], in_=ug3)

                    nc.sync.dma_start(
                        out=y[m0 + ii].rearrange("(p r) w -> p (r w)", p=128),
                        in_=Xo[:, ii * 2048 : (ii + 1) * 2048],
                    )

    if legalize:
        _legalize_waits(nc)
    return nc


def kernel(x: np.ndarray) -> np.ndarray:
    x = np.ascontiguousarray(x, dtype=np.float32)
    assert x.shape == (B, C, H, W)
    nc = build_nc()
    per = B // N_CORES
    in_maps = [
        {"x": np.ascontiguousarray(x[i * per : (i + 1) * per].reshape(IMGS_PER_CORE, H, W))}
        for i in range(N_CORES)
    ]
    res = run_bass_kernel_spmd(nc, in_maps, core_ids=list(range(N_CORES)))
    out = np.empty((B, C, H, W), dtype=np.float32)
    for i in range(N_CORES):
        out[i * per : (i + 1) * per] = res.results[i]["y"].reshape(per, C, H, W)
    return out


# revision 23
# speedup vs baseline: 1.0828x; 1.0828x over previous
"""DWT roundtrip (Haar wavedec2 x2 + band downsample -> cubic upsample + waverec2)
as a fused single-pass Trainium2 kernel.

Math: for input x, the reference computes
  aa1, lh1, hl1, hh1 = haar_dwt2(x)            # level-1 bands, [H/2, W/2]
  z = stack(haar_dwt2(aa1), area2(lh1), area2(hl1), area2(hh1))
  ...decode: aa1r = idwt2(dwt2(aa1)) == aa1 (exact roundtrip), and
  out = haar_idwt2(aa1, U(D(lh1)), U(D(hl1)), U(D(hh1)))
with U = cv2-cubic 2x upsample, D = 2x2 box mean. Everything is linear and
local, so the level-2 roundtrip cancels analytically and the whole model is

  out[2i+p, 2j+q] = P[i,j]/4 + UG_pq[i,j]        (p,q in {0,1})

where P = 2x2 block sums of x (== 2*aa1) and UG_pq = U_w(G_pq) with
  G_pq = (-1)^p DQ_lh + (-1)^q DQ_hl + (-1)^(p+q) DQ_hh,
DQ_b the 4x4-block Haar-detail sums of x (== 8*D(band_b)), and U_w the 2D cubic
upsample with the 1/16 normalization folded into its column matrix.

Layout: one image [512,512] per step; SBUF tile [128 partitions x 2048], each
partition owns 4 consecutive image rows, so every row/col pair op is a
free-axis DVE op. The quarter-res -> half-res cubic upsample runs on the
TensorEngine as Zt = G^T @ Ah^T (one matmul, no transposes needed) followed by
UG = (Zt-slice)^T @ Aw^T, and the P/4 term is accumulated into the same PSUM
via a single full-width 0.25*I matmul (P's par-major column layout matches
ug's). ScalarE interleaves PSUM results into the output tile.

VectorE is the bottleneck engine (wall-to-wall busy in the baseline trace), so
the detail chain is tuned for DVE perf modes:
 - detail ops run in bf16: contiguous tensor_tensor hits the 2x_1P packed mode
   (strided pair-ops stay 1x — stride-2 reads can't pack);
 - tensor_reduce is 1x-always on cayman, so the column reduction is a pair-sum
   tree of tensor_tensor ops in one scratch tile U, with the three level-2
   pair-sums ([q1|s1|c2] -> [DQhl|DQlh|DQhh]) merged into ONE strided TT;
 - the Hadamard combos read/write U slices placed so every op is contiguous.
Total per-image DVE: 12 ops, ~5.6us (vs ~6.9us fp32 baseline).

All four constants ship in one inline DRAM tensor (bf16 bytes, the fp32
identity stashed as 2x bf16 words) -> one DMA, issued after image 0's load so
the first RS isn't delayed by const transfers.

Sharding: pure data-parallel, batch 32 -> 4 samples (12 images) per core.
"""

import numpy as np

import concourse.bass as bass
import concourse.mybir as mybir
from concourse import tile
from concourse.bass_utils import run_bass_kernel_spmd

N_CORES = 8
B, C, H, W = 32, 3, 512, 512
IMGS_PER_CORE = (B // N_CORES) * C  # 12

F32 = mybir.dt.float32
F32R = mybir.dt.float32r
BF16 = mybir.dt.bfloat16
ADD = mybir.AluOpType.add
SUB = mybir.AluOpType.subtract

WE = (-0.03515625, 0.26171875, 0.87890625, -0.10546875)

# float32r streams 4x faster than float32 through the PE at N>=256 with
# near-fp32 accuracy (used for the aa-term identity matmuls and, when
# DETAIL_DT=F32, the upsample matmuls too).
MM_DT = F32R
DETAIL_DT = BF16  # ~4.6e-3 rel err, inside the 2e-2 gate; unlocks DVE 2x mode

# U scratch layout (element offsets, det_dt): level-1 pair fields then the
# level-2 DQ bands interleaved with the Hadamard slots so [s|lh|d] is
# contiguous for the GA/GB combo ops.
Q1_OFF, S1_OFF, C2_OFF = 0, 256, 512
DQHL, S_OFF, DQLH, D_OFF, DQHH = 768, 896, 1024, 1152, 1280
U_W = 1408


def _build_A(n):
    """Cubic 2x upsample matrix [2n, n]: out = A @ q along an axis,
    edge-replicated like cv2 (weights accumulate on clamped taps)."""
    A = np.zeros((2 * n, n), dtype=np.float64)
    Wr = (WE[3], WE[2], WE[1], WE[0])
    for u in range(n):
        for t in range(4):
            A[2 * u, min(max(u - 2 + t, 0), n - 1)] += WE[t]
            A[2 * u + 1, min(max(u - 1 + t, 0), n - 1)] += Wr[t]
    return A


def _legalize_waits(nc):
    """This walrus build accepts at most one sync wait per instruction; Tile
    occasionally emits more (notably the kernel-tail DMA drain). Hoist extra
    waits onto standalone EventSemaphore instructions placed just before."""
    for f in nc.m.functions:
        for blk in f.blocks:
            new = []
            changed = False
            for inst in blk.instructions:
                si = inst.sync_info
                if si is not None and len(si.on_wait) > 1:
                    waits = list(si.on_wait)
                    for k, w in enumerate(waits[:-1]):
                        ev = mybir.InstEventSemaphore(
                            name=f"{inst.name}_hw{k}",
                            ins=[],
                            outs=[],
                            engine=inst.engine,
                            sync_info=mybir.SyncInfo(on_wait=[w], on_update=[]),
                        )
                        new.append(ev)
                    inst.sync_info = mybir.SyncInfo(
                        on_wait=[waits[-1]], on_update=list(si.on_update)
                    )
                    changed = True
                new.append(inst)
            if changed:
                blk.instructions = new


def build_nc(n_imgs=IMGS_PER_CORE, mm_dt=MM_DT, det_dt=DETAIL_DT, legalize=True):
    nc = bass.Bass(trn_type="TRN2", target_bir_lowering=False, debug=False)

    x = nc.dram_tensor("x", [n_imgs, H, W], F32, kind="ExternalInput").ap()
    y = nc.dram_tensor("y", [n_imgs, H, W], F32, kind="ExternalOutput").ap()

    # dtype of the upsample (G-band) matmuls follows the detail chain
    g_dt = det_dt if det_dt == BF16 else mm_dt

    A = _build_A(128)
    # AhT[k, n]: n<128 -> even half-rows A[2n,k]; n>=128 -> odd half-rows.
    AhT = np.concatenate([A[0::2, :].T, A[1::2, :].T], axis=1).astype(np.float32)
    AwT = (A.T / 16.0).astype(np.float32)  # [128, 256], natural col order
    np_g = mybir.dt.np(g_dt)
    np_i4 = (0.25 * np.eye(128)).astype(np.float32)
    # One inline tensor = one const DMA: [ahT | awT | -awT | i4-as-g_dt-words].
    gw = 1 if g_dt == F32R else 2  # g_dt words per fp32
    cw = 3 * 256 + 128 * gw
    cnp = np.zeros((128, cw), dtype=np_g)
    cnp[:, 0:256] = AhT.astype(np_g)
    cnp[:, 256:512] = AwT.astype(np_g)
    cnp[:, 512:768] = (-AwT).astype(np_g)
    cnp[:, 768:cw] = np_i4.view(np.uint32 if gw == 1 else np.uint16).view(np_g)
    c_d = nc.inline_tensor(np.ascontiguousarray(cnp), name="consts").ap()

    with tile.TileContext(nc) as tc:
        with (
            tc.tile_pool(name="const", bufs=1) as cpool,
            tc.tile_pool(name="io", bufs=2) as iop,
            tc.tile_pool(name="work", bufs=4) as wp,
            tc.tile_pool(name="psum", bufs=1, space="PSUM") as pzt,
            tc.tile_pool(name="psug", bufs=1, space="PSUM") as pug,
        ):
            ct = cpool.tile([128, cw], g_dt, tag="consts")
            ahT, awT, awTn = ct[:, 0:256], ct[:, 256:512], ct[:, 512:768]
            i4_r = ct[:, 768:cw].bitcast(mm_dt)

            def g_cast(ap):
                return ap if det_dt == BF16 else ap.bitcast(mm_dt)

            import bass_rust as _br

            # Single-image steps at both ends (head: first RS only waits one
            # 1MiB load; tail: pipeline drains one image deep, not two), image
            # pairs in the middle: one DVE instruction covers both images of a
            # pair, amortizing the ~66-cycle per-op init overhead.
            # iofs staggers the PSUM slot tags of consecutive single-image
            # steps so their PE stages don't serialize on slot reuse.
            steps = (
                [(0, 1, 0), (1, 1, 1)]
                + [(2 + 2 * j, 2, 0) for j in range((n_imgs - 4) // 2)]
                + [(n_imgs - 2, 1, 0), (n_imgs - 1, 1, 1)]
            )
            for k, (m0, nm, iofs) in enumerate(steps):
                # ---- load images: partition p <- rows 4p..4p+3, m-major ----
                X = iop.tile([128, 4096], F32, tag="xin")
                nc.sync.dma_start(
                    out=X[:, 0 : nm * 2048].rearrange("p (m z) -> p m z", m=nm),
                    in_=x[m0 : m0 + nm].rearrange("m (p r) w -> p m (r w)", p=128),
                )
                if k == 0:
                    # consts go out on the scalar HWDGE ring so they overlap
                    # image 0's load on the sync ring; needed first by the
                    # zt matmuls which start ~4us into image 0's compute
                    nc.scalar.dma_start(out=ct, in_=c_d.bitcast(g_dt))
                X4 = X[:, 0 : nm * 2048].rearrange("p (m r w) -> p m r w", m=nm, r=4)

                # ---- aa path (fp32): rs = row pairs, P = 2x2 block sums ----
                RS = wp.tile([128, 2048], F32, tag="rs")
                RS4 = RS[:, 0 : nm * 1024].rearrange("p (m r w) -> p m r w", m=nm, r=2)
                nc.vector.tensor_tensor(
                    out=RS4, in0=X4[:, :, 0::2, :], in1=X4[:, :, 1::2, :], op=ADD
                )
                P = wp.tile([128, 1024], F32, tag="p")
                Pb = P.bitcast(mm_dt)
                Pb4 = Pb[:, 0 : nm * 512].rearrange("p (m r w) -> p m r w", m=nm, r=2)
                nc.vector.tensor_tensor(
                    out=Pb4, in0=RS4[:, :, :, 0::2], in1=RS4[:, :, :, 1::2], op=ADD
                )

                # ---- detail path (det_dt): e/o -> rss/rdd ----
                # One TT makes both quarter-row cross sums:
                #   e = x[4u]+x[4u+2], o = x[4u+1]+x[4u+3]
                EO = wp.tile([128, 2048], det_dt, tag="eo")
                EO4 = EO[:, 0 : nm * 1024].rearrange("p (m r w) -> p m r w", m=nm, r=2)
                nc.vector.tensor_tensor(
                    out=EO4, in0=X4[:, :, 0:2, :], in1=X4[:, :, 2:4, :], op=ADD
                )
                # rss/rdd per image; all contiguous bf16 -> DVE 2x mode
                RSD = wp.tile([128, 2048], det_dt, tag="rsd")
                RSD4 = RSD[:, 0 : nm * 1024].rearrange("p (m r w) -> p m r w", m=nm, r=2)
                nc.vector.tensor_tensor(
                    out=RSD4[:, :, 0, :], in0=EO4[:, :, 0, :], in1=EO4[:, :, 1, :], op=ADD
                )
                nc.vector.tensor_tensor(
                    out=RSD4[:, :, 1, :], in0=EO4[:, :, 0, :], in1=EO4[:, :, 1, :], op=SUB
                )
                RSDp = RSD[:, 0 : nm * 1024].rearrange(
                    "p (m r v k) -> p m r v k", m=nm, r=2, k=2
                )

                # ---- column reduction as a pair-sum tree in U ----
                # per-image U block (1536 elems, 128-granular slots):
                #  [q1(2) | s1(2) | c2(2) | hl | s | lh | d | hh | pad(2)]
                U = wp.tile([128, 3072], det_dt, tag="u")
                Us = U[:, 0 : nm * 1536]
                V128 = Us.rearrange("p (m a w) -> p m a w", m=nm, a=12)
                V256 = Us.rearrange("p (m a w) -> p m a w", m=nm, a=6)
                Um = Us.rearrange("p (m v k) -> p m v k", m=nm, v=768, k=2)
                # level 1: q1 = pairdiff(rss), c2 = pairdiff(rdd) (one TT),
                #          s1 = pairsum(rdd)
                nc.vector.tensor_tensor(
                    out=V256[:, :, 0:3:2, :],
                    in0=RSDp[:, :, :, :, 0],
                    in1=RSDp[:, :, :, :, 1],
                    op=SUB,
                )
                nc.vector.tensor_tensor(
                    out=V256[:, :, 1, :],
                    in0=RSDp[:, :, 1, :, 0],
                    in1=RSDp[:, :, 1, :, 1],
                    op=ADD,
                )
                # level 2, one merged TT: [q1|s1|c2] pairs -> [DQhl|DQlh|DQhh]
                # scattered to slots 6/8/10 so lh lands between s and d
                nc.vector.tensor_tensor(
                    out=V128[:, :, 6:11:2, :],
                    in0=Um[:, :, 0:384, 0],
                    in1=Um[:, :, 0:384, 1],
                    op=ADD,
                )

                # ---- Hadamard combos (all contiguous bf16 slices of U) ----
                #   s = hl+hh, d = hl-hh; [s|lh|d] contiguous =>
                #   GA = [lh+s | lh+d] = [G00 | -G11], GB = [lh-s | d-lh] = [G01 | G10]
                nc.vector.tensor_tensor(
                    out=V128[:, :, 7, :], in0=V128[:, :, 6, :], in1=V128[:, :, 10, :], op=ADD
                )
                nc.vector.tensor_tensor(
                    out=V128[:, :, 9, :], in0=V128[:, :, 6, :], in1=V128[:, :, 10, :], op=SUB
                )
                dql2 = _br.AP(
                    tensor=U.tensor,
                    offset=U.offset + 1024,
                    ap=[list(U.ap[0]), [1536, nm], [0, 2], [1, 128]],
                )
                GA = wp.tile([128, 512], det_dt, tag="ga")
                GA4 = GA[:, 0 : nm * 256].rearrange("p (m a w) -> p m a w", m=nm, a=2)
                nc.vector.tensor_tensor(
                    out=g_cast(GA4), in0=dql2, in1=V128[:, :, 7:10:2, :], op=ADD
                )
                GB = wp.tile([128, 512], det_dt, tag="gb")
                GB4 = GB[:, 0 : nm * 256].rearrange("p (m a w) -> p m a w", m=nm, a=2)
                nc.vector.tensor_tensor(
                    out=g_cast(GB4), in0=V128[:, :, 8:10, :], in1=V128[:, :, 7:9, :], op=SUB
                )

                Xo = iop.tile([128, 4096], F32, tag="xout")
                for ii in range(nm):
                    i = ii + iofs if nm == 1 else ii
                    # bands b0..b3 = G00, G01, G10, G11' (b3 negated; AwTn
                    # compensates)
                    go = ii * 256
                    G = [
                        GA[:, go : go + 128],
                        GB[:, go : go + 128],
                        GB[:, go + 128 : go + 256],
                        GA[:, go + 128 : go + 256],
                    ]

                    # ---- cubic upsample on PE: Zt_b = G_b^T @ AhT ----
                    zts = []
                    for pair in range(2):
                        zt_ps = pzt.tile([128, 512], F32, tag=f"zt{pair}_{i}")
                        for half in range(2):
                            nc.tensor.matmul(
                                out=zt_ps[:, half * 256 : half * 256 + 256],
                                lhsT=g_cast(G[2 * pair + half]),
                                rhs=ahT,
                                start=True,
                                stop=True,
                            )
                        zt_sb = wp.tile([128, 512], g_dt, tag=f"ztsb{pair}_{i}")
                        nc.scalar.copy(
                            out=g_cast(zt_sb) if det_dt != BF16 else zt_sb, in_=zt_ps
                        )
                        zts.append(zt_sb)

                    # ---- UG_pq = Zt-slice^T @ AwT (+ 0.25*I @ P), interleave ----
                    # P's (par, 256) column layout matches ug's, so one
                    # full-width identity matmul seeds both parity halves.
                    Xo3 = Xo[:, ii * 2048 : (ii + 1) * 2048].rearrange(
                        "p (r w) -> p r w", r=4
                    )
                    for bi, (p_par, q_par) in enumerate([(0, 0), (0, 1), (1, 0), (1, 1)]):
                        zt_sb = zts[bi // 2]
                        zoff = (bi % 2) * 256
                        rhs = awTn if bi == 3 else awT
                        ug = pug.tile([128, 512], F32, tag=f"ug{bi % 2}_{i}")
                        nc.tensor.matmul(
                            out=ug,
                            lhsT=i4_r,
                            rhs=Pb[:, ii * 512 : (ii + 1) * 512],
                            start=True,
                            stop=False,
                        )
                        for par in range(2):  # half-row parity: ev, od
                            sl = slice(par * 256, par * 256 + 256)
                            lhsT = zt_sb[:, zoff + par * 128 : zoff + par * 128 + 128]
                            nc.tensor.matmul(
                                out=ug[:, sl],
                                lhsT=g_cast(lhsT),
                                rhs=rhs,
                                start=False,
                                stop=True,
                            )
                        ug3 = ug.rearrange("p (a b) -> p a b", a=2)
                        nc.scalar.copy(out=Xo3[:, p_par::2, q_par::2], in_=ug3)

                    nc.sync.dma_start(
                        out=y[m0 + ii].rearrange("(p r) w -> p (r w)", p=128),
                        in_=Xo[:, ii * 2048 : (ii + 1) * 2048],
                    )

    if legalize:
        _legalize_waits(nc)
    return nc


def kernel(x: np.ndarray) -> np.ndarray:
    x = np.ascontiguousarray(x, dtype=np.float32)
    assert x.shape == (B, C, H, W)
    nc = build_nc()
    per = B // N_CORES
    in_maps = [
        {"x": np.ascontiguousarray(x[i * per : (i + 1) * per].reshape(IMGS_PER_CORE, H, W))}
        for i in range(N_CORES)
    ]
    res = run_bass_kernel_spmd(nc, in_maps, core_ids=list(range(N_CORES)))
    out = np.empty((B, C, H, W), dtype=np.float32)
    for i in range(N_CORES):
        out[i * per : (i + 1) * per] = res.results[i]["y"].reshape(per, C, H, W)
    return out


# revision 24
# speedup vs baseline: 1.0871x; 1.0040x over previous
"""DWT roundtrip (Haar wavedec2 x2 + band downsample -> cubic upsample + waverec2)
as a fused single-pass Trainium2 kernel.

Math: for input x, the reference computes
  aa1, lh1, hl1, hh1 = haar_dwt2(x)            # level-1 bands, [H/2, W/2]
  z = stack(haar_dwt2(aa1), area2(lh1), area2(hl1), area2(hh1))
  ...decode: aa1r = idwt2(dwt2(aa1)) == aa1 (exact roundtrip), and
  out = haar_idwt2(aa1, U(D(lh1)), U(D(hl1)), U(D(hh1)))
with U = cv2-cubic 2x upsample, D = 2x2 box mean. Everything is linear and
local, so the level-2 roundtrip cancels analytically and the whole model is

  out[2i+p, 2j+q] = P[i,j]/4 + UG_pq[i,j]        (p,q in {0,1})

where P = 2x2 block sums of x (== 2*aa1) and UG_pq = U_w(G_pq) with
  G_pq = (-1)^p DQ_lh + (-1)^q DQ_hl + (-1)^(p+q) DQ_hh,
DQ_b the 4x4-block Haar-detail sums of x (== 8*D(band_b)), and U_w the 2D cubic
upsample with the 1/16 normalization folded into its column matrix.

Layout: SBUF tiles of [128 partitions x 2048] per image, each partition owns 4
consecutive image rows, so every row/col pair op is a free-axis DVE op. The
quarter-res -> half-res cubic upsample runs on the TensorEngine as
Zt = G^T @ Ah^T (one matmul, no transposes needed) followed by
UG = (Zt-slice)^T @ Aw^T, and the P/4 term is accumulated into the same PSUM
via a single full-width 0.25*I matmul (P's par-major column layout matches
ug's). ScalarE interleaves PSUM results into the output tile.

VectorE is the bottleneck engine (wall-to-wall busy in the baseline trace), so
the detail chain is tuned for DVE perf modes:
 - detail ops run in bf16: contiguous tensor_tensor hits the 2x_1P packed mode
   (strided pair-ops stay 1x — stride-2 reads can't pack);
 - tensor_reduce is 1x-always on cayman, so the column reduction is a pair-sum
   tree of tensor_tensor ops in one scratch tile U, with the three level-2
   pair-sums ([q1|s1|c2] -> [DQhl|DQlh|DQhh]) merged into ONE strided TT;
 - the Hadamard combos read/write U slices placed so every op is contiguous;
 - the middle images are processed two per DVE instruction (steps list),
   amortizing the ~66-cycle per-op init overhead; single-image steps cap both
   ends so the pipeline fills after one 1MiB load and drains one image deep.

All four constants ship in one inline DRAM tensor (bf16 bytes, the fp32
identity stashed as 2x bf16 words) -> one DMA on the scalar HWDGE ring so it
overlaps image 0's load on the sync ring.

Negative results worth remembering (all measured slower):
 - GpSimd tensor_tensor for any stage (EO, P): ~2.5 cyc/elem Q7 floor plus
   SBUF contention with the DVE costs 10-25us even off the critical path;
 - identity matmuls with strided rhs reading RS directly (PE f32r streams
   ~2 cyc/col, and +8 matmuls/img serialize the PE past the DVE);
 - out-DMAs on the scalar ring or split per row-parity: DMA issue occupancy
   on the ACT sequencer / sync-ring FIFO stalls outweigh the overlap gained.

Sharding: pure data-parallel, batch 32 -> 4 samples (12 images) per core.
"""

import numpy as np

import concourse.bass as bass
import concourse.mybir as mybir
from concourse import tile
from concourse.bass_utils import run_bass_kernel_spmd

N_CORES = 8
B, C, H, W = 32, 3, 512, 512
IMGS_PER_CORE = (B // N_CORES) * C  # 12

F32 = mybir.dt.float32
F32R = mybir.dt.float32r
BF16 = mybir.dt.bfloat16
ADD = mybir.AluOpType.add
SUB = mybir.AluOpType.subtract

WE = (-0.03515625, 0.26171875, 0.87890625, -0.10546875)

# float32r streams 4x faster than float32 through the PE at N>=256 with
# near-fp32 accuracy (used for the aa-term identity matmuls and, when
# DETAIL_DT=F32, the upsample matmuls too).
MM_DT = F32R
DETAIL_DT = BF16  # ~4.6e-3 rel err, inside the 2e-2 gate; unlocks DVE 2x mode

# U scratch layout (element offsets, det_dt): level-1 pair fields then the
# level-2 DQ bands interleaved with the Hadamard slots so [s|lh|d] is
# contiguous for the GA/GB combo ops.
Q1_OFF, S1_OFF, C2_OFF = 0, 256, 512
DQHL, S_OFF, DQLH, D_OFF, DQHH = 768, 896, 1024, 1152, 1280
U_W = 1408


def _build_A(n):
    """Cubic 2x upsample matrix [2n, n]: out = A @ q along an axis,
    edge-replicated like cv2 (weights accumulate on clamped taps)."""
    A = np.zeros((2 * n, n), dtype=np.float64)
    Wr = (WE[3], WE[2], WE[1], WE[0])
    for u in range(n):
        for t in range(4):
            A[2 * u, min(max(u - 2 + t, 0), n - 1)] += WE[t]
            A[2 * u + 1, min(max(u - 1 + t, 0), n - 1)] += Wr[t]
    return A


def _legalize_waits(nc):
    """This walrus build accepts at most one sync wait per instruction; Tile
    occasionally emits more (notably the kernel-tail DMA drain). Hoist extra
    waits onto standalone EventSemaphore instructions placed just before."""
    for f in nc.m.functions:
        for blk in f.blocks:
            new = []
            changed = False
            for inst in blk.instructions:
                si = inst.sync_info
                if si is not None and len(si.on_wait) > 1:
                    waits = list(si.on_wait)
                    for k, w in enumerate(waits[:-1]):
                        ev = mybir.InstEventSemaphore(
                            name=f"{inst.name}_hw{k}",
                            ins=[],
                            outs=[],
                            engine=inst.engine,
                            sync_info=mybir.SyncInfo(on_wait=[w], on_update=[]),
                        )
                        new.append(ev)
                    inst.sync_info = mybir.SyncInfo(
                        on_wait=[waits[-1]], on_update=list(si.on_update)
                    )
                    changed = True
                new.append(inst)
            if changed:
                blk.instructions = new


def build_nc(n_imgs=IMGS_PER_CORE, mm_dt=MM_DT, det_dt=DETAIL_DT, legalize=True):
    nc = bass.Bass(trn_type="TRN2", target_bir_lowering=False, debug=False)

    x = nc.dram_tensor("x", [n_imgs, H, W], F32, kind="ExternalInput").ap()
    y = nc.dram_tensor("y", [n_imgs, H, W], F32, kind="ExternalOutput").ap()

    # dtype of the upsample (G-band) matmuls follows the detail chain
    g_dt = det_dt if det_dt == BF16 else mm_dt

    A = _build_A(128)
    # AhT[k, n]: n<128 -> even half-rows A[2n,k]; n>=128 -> odd half-rows.
    AhT = np.concatenate([A[0::2, :].T, A[1::2, :].T], axis=1).astype(np.float32)
    AwT = (A.T / 16.0).astype(np.float32)  # [128, 256], natural col order
    np_g = mybir.dt.np(g_dt)
    np_i4 = (0.25 * np.eye(128)).astype(np.float32)
    # One inline tensor = one const DMA: [ahT | awT | -awT | i4-as-g_dt-words].
    gw = 1 if g_dt == F32R else 2  # g_dt words per fp32
    cw = 3 * 256 + 128 * gw
    cnp = np.zeros((128, cw), dtype=np_g)
    cnp[:, 0:256] = AhT.astype(np_g)
    cnp[:, 256:512] = AwT.astype(np_g)
    cnp[:, 512:768] = (-AwT).astype(np_g)
    cnp[:, 768:cw] = np_i4.view(np.uint32 if gw == 1 else np.uint16).view(np_g)
    c_d = nc.inline_tensor(np.ascontiguousarray(cnp), name="consts").ap()

    with tile.TileContext(nc) as tc:
        with (
            tc.tile_pool(name="const", bufs=1) as cpool,
            tc.tile_pool(name="io", bufs=2) as iop,
            tc.tile_pool(name="work", bufs=4) as wp,
            tc.tile_pool(name="psum", bufs=1, space="PSUM") as pzt,
            tc.tile_pool(name="psug", bufs=1, space="PSUM") as pug,
        ):
            ct = cpool.tile([128, cw], g_dt, tag="consts")
            ahT, awT, awTn = ct[:, 0:256], ct[:, 256:512], ct[:, 512:768]
            i4_r = ct[:, 768:cw].bitcast(mm_dt)

            def g_cast(ap):
                return ap if det_dt == BF16 else ap.bitcast(mm_dt)

            import bass_rust as _br

            # Single-image steps at both ends (head: first RS only waits one
            # 1MiB load; tail: pipeline drains one image deep, not two), image
            # pairs in the middle: one DVE instruction covers both images of a
            # pair, amortizing the ~66-cycle per-op init overhead.
            # iofs staggers the PSUM slot tags of consecutive single-image
            # steps so their PE stages don't serialize on slot reuse.
            steps = (
                [(0, 1, 0), (1, 1, 1)]
                + [(2 + 2 * j, 2, 0) for j in range((n_imgs - 4) // 2)]
                + [(n_imgs - 2, 1, 0), (n_imgs - 1, 1, 1)]
            )
            for k, (m0, nm, iofs) in enumerate(steps):
                # ---- load images: partition p <- rows 4p..4p+3, m-major ----
                X = iop.tile([128, 4096], F32, tag="xin")
                nc.sync.dma_start(
                    out=X[:, 0 : nm * 2048].rearrange("p (m z) -> p m z", m=nm),
                    in_=x[m0 : m0 + nm].rearrange("m (p r) w -> p m (r w)", p=128),
                )
                if k == 0:
                    # consts go out on the scalar HWDGE ring so they overlap
                    # image 0's load on the sync ring; needed first by the
                    # zt matmuls which start ~4us into image 0's compute
                    nc.scalar.dma_start(out=ct, in_=c_d.bitcast(g_dt))
                X4 = X[:, 0 : nm * 2048].rearrange("p (m r w) -> p m r w", m=nm, r=4)

                # ---- aa path (fp32): rs = row pairs, P = 2x2 block sums ----
                RS = wp.tile([128, 2048], F32, tag="rs")
                RS4 = RS[:, 0 : nm * 1024].rearrange("p (m r w) -> p m r w", m=nm, r=2)
                nc.vector.tensor_tensor(
                    out=RS4, in0=X4[:, :, 0::2, :], in1=X4[:, :, 1::2, :], op=ADD
                )
                P = wp.tile([128, 1024], F32, tag="p")
                Pb = P.bitcast(mm_dt)
                Pb4 = Pb[:, 0 : nm * 512].rearrange("p (m r w) -> p m r w", m=nm, r=2)
                nc.vector.tensor_tensor(
                    out=Pb4, in0=RS4[:, :, :, 0::2], in1=RS4[:, :, :, 1::2], op=ADD
                )

                # ---- detail path (det_dt): e/o -> rss/rdd ----
                # One TT makes both quarter-row cross sums:
                #   e = x[4u]+x[4u+2], o = x[4u+1]+x[4u+3]
                EO = wp.tile([128, 2048], det_dt, tag="eo")
                EO4 = EO[:, 0 : nm * 1024].rearrange("p (m r w) -> p m r w", m=nm, r=2)
                nc.vector.tensor_tensor(
                    out=EO4, in0=X4[:, :, 0:2, :], in1=X4[:, :, 2:4, :], op=ADD
                )
                # rss/rdd per image; all contiguous bf16 -> DVE 2x mode
                RSD = wp.tile([128, 2048], det_dt, tag="rsd")
                RSD4 = RSD[:, 0 : nm * 1024].rearrange("p (m r w) -> p m r w", m=nm, r=2)
                nc.vector.tensor_tensor(
                    out=RSD4[:, :, 0, :], in0=EO4[:, :, 0, :], in1=EO4[:, :, 1, :], op=ADD
                )
                nc.vector.tensor_tensor(
                    out=RSD4[:, :, 1, :], in0=EO4[:, :, 0, :], in1=EO4[:, :, 1, :], op=SUB
                )
                RSDp = RSD[:, 0 : nm * 1024].rearrange(
                    "p (m r v k) -> p m r v k", m=nm, r=2, k=2
                )

                # ---- column reduction as a pair-sum tree in U ----
                # per-image U block (1536 elems, 128-granular slots):
                #  [q1(2) | s1(2) | c2(2) | hl | s | lh | d | hh | pad(2)]
                U = wp.tile([128, 3072], det_dt, tag="u")
                Us = U[:, 0 : nm * 1536]
                V128 = Us.rearrange("p (m a w) -> p m a w", m=nm, a=12)
                V256 = Us.rearrange("p (m a w) -> p m a w", m=nm, a=6)
                Um = Us.rearrange("p (m v k) -> p m v k", m=nm, v=768, k=2)
                # level 1: q1 = pairdiff(rss), c2 = pairdiff(rdd) (one TT),
                #          s1 = pairsum(rdd)
                nc.vector.tensor_tensor(
                    out=V256[:, :, 0:3:2, :],
                    in0=RSDp[:, :, :, :, 0],
                    in1=RSDp[:, :, :, :, 1],
                    op=SUB,
                )
                nc.vector.tensor_tensor(
                    out=V256[:, :, 1, :],
                    in0=RSDp[:, :, 1, :, 0],
                    in1=RSDp[:, :, 1, :, 1],
                    op=ADD,
                )
                # level 2, one merged TT: [q1|s1|c2] pairs -> [DQhl|DQlh|DQhh]
                # scattered to slots 6/8/10 so lh lands between s and d
                nc.vector.tensor_tensor(
                    out=V128[:, :, 6:11:2, :],
                    in0=Um[:, :, 0:384, 0],
                    in1=Um[:, :, 0:384, 1],
                    op=ADD,
                )

                # ---- Hadamard combos (all contiguous bf16 slices of U) ----
                #   s = hl+hh, d = hl-hh; [s|lh|d] contiguous =>
                #   GA = [lh+s | lh+d] = [G00 | -G11], GB = [lh-s | d-lh] = [G01 | G10]
                nc.vector.tensor_tensor(
                    out=V128[:, :, 7, :], in0=V128[:, :, 6, :], in1=V128[:, :, 10, :], op=ADD
                )
                nc.vector.tensor_tensor(
                    out=V128[:, :, 9, :], in0=V128[:, :, 6, :], in1=V128[:, :, 10, :], op=SUB
                )
                dql2 = _br.AP(
                    tensor=U.tensor,
                    offset=U.offset + 1024,
                    ap=[list(U.ap[0]), [1536, nm], [0, 2], [1, 128]],
                )
                GA = wp.tile([128, 512], det_dt, tag="ga")
                GA4 = GA[:, 0 : nm * 256].rearrange("p (m a w) -> p m a w", m=nm, a=2)
                nc.vector.tensor_tensor(
                    out=g_cast(GA4), in0=dql2, in1=V128[:, :, 7:10:2, :], op=ADD
                )
                GB = wp.tile([128, 512], det_dt, tag="gb")
                GB4 = GB[:, 0 : nm * 256].rearrange("p (m a w) -> p m a w", m=nm, a=2)
                nc.vector.tensor_tensor(
                    out=g_cast(GB4), in0=V128[:, :, 8:10, :], in1=V128[:, :, 7:9, :], op=SUB
                )

                Xo = iop.tile([128, 4096], F32, tag="xout")
                for ii in range(nm):
                    i = ii + iofs if nm == 1 else ii
                    # bands b0..b3 = G00, G01, G10, G11' (b3 negated; AwTn
                    # compensates)
                    go = ii * 256
                    G = [
                        GA[:, go : go + 128],
                        GB[:, go : go + 128],
                        GB[:, go + 128 : go + 256],
                        GA[:, go + 128 : go + 256],
                    ]

                    # ---- cubic upsample on PE: Zt_b = G_b^T @ AhT ----
                    zts = []
                    for pair in range(2):
                        zt_ps = pzt.tile([128, 512], F32, tag=f"zt{pair}_{i}")
                        for half in range(2):
                            nc.tensor.matmul(
                                out=zt_ps[:, half * 256 : half * 256 + 256],
                                lhsT=g_cast(G[2 * pair + half]),
                                rhs=ahT,
                                start=True,
                                stop=True,
                            )
                        zt_sb = wp.tile([128, 512], g_dt, tag=f"ztsb{pair}_{i}")
                        nc.scalar.copy(
                            out=g_cast(zt_sb) if det_dt != BF16 else zt_sb, in_=zt_ps
                        )
                        zts.append(zt_sb)

                    # ---- UG_pq = Zt-slice^T @ AwT (+ 0.25*I @ P), interleave ----
                    # P's (par, 256) column layout matches ug's, so one
                    # full-width identity matmul seeds both parity halves.
                    Xo3 = Xo[:, ii * 2048 : (ii + 1) * 2048].rearrange(
                        "p (r w) -> p r w", r=4
                    )
                    for bi, (p_par, q_par) in enumerate([(0, 0), (0, 1), (1, 0), (1, 1)]):
                        zt_sb = zts[bi // 2]
                        zoff = (bi % 2) * 256
                        rhs = awTn if bi == 3 else awT
                        ug = pug.tile([128, 512], F32, tag=f"ug{bi % 2}_{i}")
                        nc.tensor.matmul(
                            out=ug,
                            lhsT=i4_r,
                            rhs=Pb[:, ii * 512 : (ii + 1) * 512],
                            start=True,
                            stop=False,
                        )
                        for par in range(2):  # half-row parity: ev, od
                            sl = slice(par * 256, par * 256 + 256)
                            lhsT = zt_sb[:, zoff + par * 128 : zoff + par * 128 + 128]
                            nc.tensor.matmul(
                                out=ug[:, sl],
                                lhsT=g_cast(lhsT),
                                rhs=rhs,
                                start=False,
                                stop=True,
                            )
                        ug3 = ug.rearrange("p (a b) -> p a b", a=2)
                        nc.scalar.copy(out=Xo3[:, p_par::2, q_par::2], in_=ug3)

                    nc.sync.dma_start(
                        out=y[m0 + ii].rearrange("(p r) w -> p (r w)", p=128),
                        in_=Xo[:, ii * 2048 : (ii + 1) * 2048],
                    )

    if legalize:
        _legalize_waits(nc)
    return nc


def kernel(x: np.ndarray) -> np.ndarray:
    x = np.ascontiguousarray(x, dtype=np.float32)
    assert x.shape == (B, C, H, W)
    nc = build_nc()
    per = B // N_CORES
    in_maps = [
        {"x": np.ascontiguousarray(x[i * per : (i + 1) * per].reshape(IMGS_PER_CORE, H, W))}
        for i in range(N_CORES)
    ]
    res = run_bass_kernel_spmd(nc, in_maps, core_ids=list(range(N_CORES)))
    out = np.empty((B, C, H, W), dtype=np.float32)
    for i in range(N_CORES):
        out[i * per : (i + 1) * per] = res.results[i]["y"].reshape(per, C, H, W)
    return out


# revision 25
# speedup vs baseline: 1.1289x; 1.0384x over previous
"""DWT roundtrip (Haar wavedec2 x2 + band downsample -> cubic upsample + waverec2)
as a fused single-pass Trainium2 kernel.

Math: for input x, the reference computes
  aa1, lh1, hl1, hh1 = haar_dwt2(x)            # level-1 bands, [H/2, W/2]
  z = stack(haar_dwt2(aa1), area2(lh1), area2(hl1), area2(hh1))
  ...decode: aa1r = idwt2(dwt2(aa1)) == aa1 (exact roundtrip), and
  out = haar_idwt2(aa1, U(D(lh1)), U(D(hl1)), U(D(hh1)))
with U = cv2-cubic 2x upsample, D = 2x2 box mean. Everything is linear and
local, so the level-2 roundtrip cancels analytically and the whole model is

  out[2i+p, 2j+q] = P[i,j]/4 + UG_pq[i,j]        (p,q in {0,1})

where P = 2x2 block sums of x (== 2*aa1) and UG_pq = U_w(G_pq) with
  G_pq = (-1)^p DQ_lh + (-1)^q DQ_hl + (-1)^(p+q) DQ_hh,
DQ_b the 4x4-block Haar-detail sums of x (== 8*D(band_b)), and U_w the 2D cubic
upsample with the 1/16 normalization folded into its column matrix.

Layout: SBUF tiles of [128 partitions x 2048] per image, each partition owns 4
consecutive image rows, so every row/col pair op is a free-axis DVE op. The
quarter-res -> half-res cubic upsample runs on the TensorEngine as
Zt = G^T @ Ah^T (one matmul, no transposes needed) followed by
UG = (Zt-slice)^T @ Aw^T, and the P/4 term is accumulated into the same PSUM
via a single full-width 0.25*I matmul (P's par-major column layout matches
ug's). ScalarE interleaves PSUM results into the output tile.

VectorE is the bottleneck engine (wall-to-wall busy in the baseline trace), so
the detail chain is tuned for DVE perf modes:
 - detail ops run in bf16: contiguous tensor_tensor hits the 2x_1P packed mode
   (strided pair-ops stay 1x — stride-2 reads can't pack);
 - tensor_reduce is 1x-always on cayman, so the column reduction is a pair-sum
   tree of tensor_tensor ops in one scratch tile U, with the three level-2
   pair-sums ([q1|s1|c2] -> [DQhl|DQlh|DQhh]) merged into ONE strided TT;
 - the Hadamard combos read/write U slices placed so every op is contiguous;
 - the middle images are processed two per DVE instruction (steps list),
   amortizing the ~66-cycle per-op init overhead; single-image steps cap both
   ends so the pipeline fills after one 1MiB load and drains one image deep.

All four constants ship in one inline DRAM tensor (bf16 bytes, the fp32
identity stashed as 2x bf16 words) -> one DMA on the scalar HWDGE ring so it
overlaps image 0's load on the sync ring.

Negative results worth remembering (all measured slower):
 - GpSimd tensor_tensor for any stage (EO, P): ~2.5 cyc/elem Q7 floor plus
   SBUF contention with the DVE costs 10-25us even off the critical path;
 - identity matmuls with strided rhs reading RS directly (PE f32r streams
   ~2 cyc/col, and +8 matmuls/img serialize the PE past the DVE);
 - out-DMAs on the scalar ring or split per row-parity: DMA issue occupancy
   on the ACT sequencer / sync-ring FIFO stalls outweigh the overlap gained.

Sharding: pure data-parallel, batch 32 -> 4 samples (12 images) per core.
"""

import numpy as np

import concourse.bass as bass
import concourse.mybir as mybir
from concourse import tile
from concourse.bass_utils import run_bass_kernel_spmd

N_CORES = 8
B, C, H, W = 32, 3, 512, 512
IMGS_PER_CORE = (B // N_CORES) * C  # 12

F32 = mybir.dt.float32
F32R = mybir.dt.float32r
BF16 = mybir.dt.bfloat16
ADD = mybir.AluOpType.add
SUB = mybir.AluOpType.subtract

WE = (-0.03515625, 0.26171875, 0.87890625, -0.10546875)

# float32r streams 4x faster than float32 through the PE at N>=256 with
# near-fp32 accuracy (used for the aa-term identity matmuls and, when
# DETAIL_DT=F32, the upsample matmuls too).
MM_DT = F32R
DETAIL_DT = BF16  # ~4.6e-3 rel err, inside the 2e-2 gate; unlocks DVE 2x mode

# U scratch layout (element offsets, det_dt): level-1 pair fields then the
# level-2 DQ bands interleaved with the Hadamard slots so [s|lh|d] is
# contiguous for the GA/GB combo ops.
Q1_OFF, S1_OFF, C2_OFF = 0, 256, 512
DQHL, S_OFF, DQLH, D_OFF, DQHH = 768, 896, 1024, 1152, 1280
U_W = 1408


def _build_A(n):
    """Cubic 2x upsample matrix [2n, n]: out = A @ q along an axis,
    edge-replicated like cv2 (weights accumulate on clamped taps)."""
    A = np.zeros((2 * n, n), dtype=np.float64)
    Wr = (WE[3], WE[2], WE[1], WE[0])
    for u in range(n):
        for t in range(4):
            A[2 * u, min(max(u - 2 + t, 0), n - 1)] += WE[t]
            A[2 * u + 1, min(max(u - 1 + t, 0), n - 1)] += Wr[t]
    return A


def _legalize_waits(nc):
    """This walrus build accepts at most one sync wait per instruction; Tile
    occasionally emits more (notably the kernel-tail DMA drain). Hoist extra
    waits onto standalone EventSemaphore instructions placed just before."""
    for f in nc.m.functions:
        for blk in f.blocks:
            new = []
            changed = False
            for inst in blk.instructions:
                si = inst.sync_info
                if si is not None and len(si.on_wait) > 1:
                    waits = list(si.on_wait)
                    for k, w in enumerate(waits[:-1]):
                        ev = mybir.InstEventSemaphore(
                            name=f"{inst.name}_hw{k}",
                            ins=[],
                            outs=[],
                            engine=inst.engine,
                            sync_info=mybir.SyncInfo(on_wait=[w], on_update=[]),
                        )
                        new.append(ev)
                    inst.sync_info = mybir.SyncInfo(
                        on_wait=[waits[-1]], on_update=list(si.on_update)
                    )
                    changed = True
                new.append(inst)
            if changed:
                blk.instructions = new


def build_nc(n_imgs=IMGS_PER_CORE, mm_dt=MM_DT, det_dt=DETAIL_DT, legalize=True):
    nc = bass.Bass(trn_type="TRN2", target_bir_lowering=False, debug=False)

    x = nc.dram_tensor("x", [n_imgs, H, W], F32, kind="ExternalInput").ap()
    y = nc.dram_tensor("y", [n_imgs, H, W], F32, kind="ExternalOutput").ap()

    # dtype of the upsample (G-band) matmuls follows the detail chain
    g_dt = det_dt if det_dt == BF16 else mm_dt

    A = _build_A(128)
    # AhT[k, n]: n<128 -> even half-rows A[2n,k]; n>=128 -> odd half-rows.
    AhT = np.concatenate([A[0::2, :].T, A[1::2, :].T], axis=1).astype(np.float32)
    AwT = (A.T / 16.0).astype(np.float32)  # [128, 256], natural col order
    np_g = mybir.dt.np(g_dt)
    np_i4 = (0.25 * np.eye(128)).astype(np.float32)
    # One inline tensor = one const DMA: [ahT | awT | -awT | i4-as-g_dt-words].
    gw = 1 if g_dt == F32R else 2  # g_dt words per fp32
    cw = 3 * 256 + 128 * gw
    cnp = np.zeros((128, cw), dtype=np_g)
    cnp[:, 0:256] = AhT.astype(np_g)
    cnp[:, 256:512] = AwT.astype(np_g)
    cnp[:, 512:768] = (-AwT).astype(np_g)
    cnp[:, 768:cw] = np_i4.view(np.uint32 if gw == 1 else np.uint16).view(np_g)
    c_d = nc.inline_tensor(np.ascontiguousarray(cnp), name="consts").ap()

    with tile.TileContext(nc) as tc:
        with (
            tc.tile_pool(name="const", bufs=1) as cpool,
            tc.tile_pool(name="io", bufs=2) as iop,
            tc.tile_pool(name="work", bufs=4) as wp,
            tc.tile_pool(name="psum", bufs=1, space="PSUM") as pzt,
            tc.tile_pool(name="psug", bufs=1, space="PSUM") as pug,
        ):
            ct = cpool.tile([128, cw], g_dt, tag="consts")
            ahT, awT, awTn = ct[:, 0:256], ct[:, 256:512], ct[:, 512:768]
            i4_r = ct[:, 768:cw].bitcast(mm_dt)

            def g_cast(ap):
                return ap if det_dt == BF16 else ap.bitcast(mm_dt)

            import bass_rust as _br

            # Single-image steps at both ends (head: first RS only waits one
            # 1MiB load; tail: pipeline drains one image deep, not two), image
            # pairs in the middle: one DVE instruction covers both images of a
            # pair, amortizing the ~66-cycle per-op init overhead.
            steps = (
                [(0, 1, 0), (1, 1, 0)]
                + [(2 + 2 * j, 2, 0) for j in range((n_imgs - 4) // 2)]
                + [(n_imgs - 2, 1, 0), (n_imgs - 1, 1, 0)]
            )
            for k, (m0, nm, iofs) in enumerate(steps):
                # ---- load images: partition p <- rows 4p..4p+3, m-major ----
                X = iop.tile([128, 4096], F32, tag="xin")
                nc.sync.dma_start(
                    out=X[:, 0 : nm * 2048].rearrange("p (m z) -> p m z", m=nm),
                    in_=x[m0 : m0 + nm].rearrange("m (p r) w -> p m (r w)", p=128),
                )
                if k == 0:
                    # consts go out on the scalar HWDGE ring so they overlap
                    # image 0's load on the sync ring; needed first by the
                    # zt matmuls which start ~4us into image 0's compute
                    nc.scalar.dma_start(out=ct, in_=c_d.bitcast(g_dt))
                X4 = X[:, 0 : nm * 2048].rearrange("p (m r w) -> p m r w", m=nm, r=4)

                # ---- aa path (fp32): rs = row pairs, P = 2x2 block sums ----
                RS = wp.tile([128, 2048], F32, tag="rs")
                RS4 = RS[:, 0 : nm * 1024].rearrange("p (m r w) -> p m r w", m=nm, r=2)
                nc.vector.tensor_tensor(
                    out=RS4, in0=X4[:, :, 0::2, :], in1=X4[:, :, 1::2, :], op=ADD
                )
                P = wp.tile([128, 1024], F32, tag="p")
                Pb = P.bitcast(mm_dt)
                Pb4 = Pb[:, 0 : nm * 512].rearrange("p (m r w) -> p m r w", m=nm, r=2)
                nc.vector.tensor_tensor(
                    out=Pb4, in0=RS4[:, :, :, 0::2], in1=RS4[:, :, :, 1::2], op=ADD
                )

                # ---- detail path (det_dt): e/o -> rss/rdd ----
                # One TT makes both quarter-row cross sums:
                #   e = x[4u]+x[4u+2], o = x[4u+1]+x[4u+3]
                EO = wp.tile([128, 2048], det_dt, tag="eo")
                EO4 = EO[:, 0 : nm * 1024].rearrange("p (m r w) -> p m r w", m=nm, r=2)
                nc.vector.tensor_tensor(
                    out=EO4, in0=X4[:, :, 0:2, :], in1=X4[:, :, 2:4, :], op=ADD
                )
                # rss/rdd per image; all contiguous bf16 -> DVE 2x mode
                RSD = wp.tile([128, 2048], det_dt, tag="rsd")
                RSD4 = RSD[:, 0 : nm * 1024].rearrange("p (m r w) -> p m r w", m=nm, r=2)
                nc.vector.tensor_tensor(
                    out=RSD4[:, :, 0, :], in0=EO4[:, :, 0, :], in1=EO4[:, :, 1, :], op=ADD
                )
                nc.vector.tensor_tensor(
                    out=RSD4[:, :, 1, :], in0=EO4[:, :, 0, :], in1=EO4[:, :, 1, :], op=SUB
                )
                RSDp = RSD[:, 0 : nm * 1024].rearrange(
                    "p (m r v k) -> p m r v k", m=nm, r=2, k=2
                )

                # ---- column reduction as a pair-sum tree in U ----
                # per-image U block (1536 elems, 128-granular slots):
                #  [q1(2) | s1(2) | c2(2) | hl | s | lh | d | hh | pad(2)]
                U = wp.tile([128, 3072], det_dt, tag="u")
                Us = U[:, 0 : nm * 1536]
                V128 = Us.rearrange("p (m a w) -> p m a w", m=nm, a=12)
                V256 = Us.rearrange("p (m a w) -> p m a w", m=nm, a=6)
                Um = Us.rearrange("p (m v k) -> p m v k", m=nm, v=768, k=2)
                # level 1: q1 = pairdiff(rss), c2 = pairdiff(rdd) (one TT),
                #          s1 = pairsum(rdd)
                nc.vector.tensor_tensor(
                    out=V256[:, :, 0:3:2, :],
                    in0=RSDp[:, :, :, :, 0],
                    in1=RSDp[:, :, :, :, 1],
                    op=SUB,
                )
                nc.vector.tensor_tensor(
                    out=V256[:, :, 1, :],
                    in0=RSDp[:, :, 1, :, 0],
                    in1=RSDp[:, :, 1, :, 1],
                    op=ADD,
                )
                # level 2, one merged TT: [q1|s1|c2] pairs -> [DQhl|DQlh|DQhh]
                # scattered to slots 6/8/10 so lh lands between s and d
                nc.vector.tensor_tensor(
                    out=V128[:, :, 6:11:2, :],
                    in0=Um[:, :, 0:384, 0],
                    in1=Um[:, :, 0:384, 1],
                    op=ADD,
                )

                # ---- Hadamard combos (all contiguous bf16 slices of U) ----
                #   s = hl+hh, d = hl-hh; [s|lh|d] contiguous =>
                #   GA = [lh+s | lh+d] = [G00 | -G11], GB = [lh-s | d-lh] = [G01 | G10]
                nc.vector.tensor_tensor(
                    out=V128[:, :, 7, :], in0=V128[:, :, 6, :], in1=V128[:, :, 10, :], op=ADD
                )
                nc.vector.tensor_tensor(
                    out=V128[:, :, 9, :], in0=V128[:, :, 6, :], in1=V128[:, :, 10, :], op=SUB
                )
                dql2 = _br.AP(
                    tensor=U.tensor,
                    offset=U.offset + 1024,
                    ap=[list(U.ap[0]), [1536, nm], [0, 2], [1, 128]],
                )
                GA = wp.tile([128, 512], det_dt, tag="ga")
                GA4 = GA[:, 0 : nm * 256].rearrange("p (m a w) -> p m a w", m=nm, a=2)
                nc.vector.tensor_tensor(
                    out=g_cast(GA4), in0=dql2, in1=V128[:, :, 7:10:2, :], op=ADD
                )
                GB = wp.tile([128, 512], det_dt, tag="gb")
                GB4 = GB[:, 0 : nm * 256].rearrange("p (m a w) -> p m a w", m=nm, a=2)
                nc.vector.tensor_tensor(
                    out=g_cast(GB4), in0=V128[:, :, 8:10, :], in1=V128[:, :, 7:9, :], op=SUB
                )

                Xo = iop.tile([128, 4096], F32, tag="xout")
                for ii in range(nm):
                    i = ii + iofs if nm == 1 else ii
                    # bands b0..b3 = G00, G01, G10, G11' (b3 negated; AwTn
                    # compensates)
                    go = ii * 256
                    G = [
                        GA[:, go : go + 128],
                        GB[:, go : go + 128],
                        GB[:, go + 128 : go + 256],
                        GA[:, go + 128 : go + 256],
                    ]

                    # ---- cubic upsample on PE: Zt_b = G_b^T @ AhT ----
                    zts = []
                    for pair in range(2):
                        zt_ps = pzt.tile([128, 512], F32, tag=f"zt{pair}_{i}")
                        for half in range(2):
                            nc.tensor.matmul(
                                out=zt_ps[:, half * 256 : half * 256 + 256],
                                lhsT=g_cast(G[2 * pair + half]),
                                rhs=ahT,
                                start=True,
                                stop=True,
                            )
                        zt_sb = wp.tile([128, 512], g_dt, tag=f"ztsb{pair}_{i}")
                        nc.scalar.copy(
                            out=g_cast(zt_sb) if det_dt != BF16 else zt_sb, in_=zt_ps
                        )
                        zts.append(zt_sb)

                    # ---- UG_pq = Zt-slice^T @ AwT (+ 0.25*I @ P), interleave ----
                    # P's (par, 256) column layout matches ug's, so one
                    # full-width identity matmul seeds both parity halves.
                    Xo3 = Xo[:, ii * 2048 : (ii + 1) * 2048].rearrange(
                        "p (r w) -> p r w", r=4
                    )
                    for bi, (p_par, q_par) in enumerate([(0, 0), (0, 1), (1, 0), (1, 1)]):
                        zt_sb = zts[bi // 2]
                        zoff = (bi % 2) * 256
                        rhs = awTn if bi == 3 else awT
                        ug = pug.tile([128, 512], F32, tag=f"ug{bi % 2}_{i}")
                        nc.tensor.matmul(
                            out=ug,
                            lhsT=i4_r,
                            rhs=Pb[:, ii * 512 : (ii + 1) * 512],
                            start=True,
                            stop=False,
                        )
                        for par in range(2):  # half-row parity: ev, od
                            sl = slice(par * 256, par * 256 + 256)
                            lhsT = zt_sb[:, zoff + par * 128 : zoff + par * 128 + 128]
                            nc.tensor.matmul(
                                out=ug[:, sl],
                                lhsT=g_cast(lhsT),
                                rhs=rhs,
                                start=False,
                                stop=True,
                            )
                        ug3 = ug.rearrange("p (a b) -> p a b", a=2)
                        nc.scalar.copy(out=Xo3[:, p_par::2, q_par::2], in_=ug3)

                    nc.sync.dma_start(
                        out=y[m0 + ii].rearrange("(p r) w -> p (r w)", p=128),
                        in_=Xo[:, ii * 2048 : (ii + 1) * 2048],
                    )

    if legalize:
        _legalize_waits(nc)
    return nc


def kernel(x: np.ndarray) -> np.ndarray:
    x = np.ascontiguousarray(x, dtype=np.float32)
    assert x.shape == (B, C, H, W)
    nc = build_nc()
    per = B // N_CORES
    in_maps = [
        {"x": np.ascontiguousarray(x[i * per : (i + 1) * per].reshape(IMGS_PER_CORE, H, W))}
        for i in range(N_CORES)
    ]
    res = run_bass_kernel_spmd(nc, in_maps, core_ids=list(range(N_CORES)))
    out = np.empty((B, C, H, W), dtype=np.float32)
    for i in range(N_CORES):
        out[i * per : (i + 1) * per] = res.results[i]["y"].reshape(per, C, H, W)
    return out
